# revision 26
# baseline (speedup 1.0000x reference)
"""External Attention (nn_External_Attention) on 8 TRN2 NeuronCores.

kernel(x, Wk, Wv) -> x + Wv @ l1norm_M(softmax_N(Wk @ x))
  x  [16, 512, 4096] f32,  Wk [256, 512] f32,  Wv [512, 256] f32

Sharding: data-parallel over batch B=16 -> 2 batches per core across 8 cores.
Each core runs an identical Bass/Tile program on its batch shard; results are
concatenated on host.

Per-core pipeline (C=512, M=256, N=4096), all-bf16 on the PE:
  x is loaded HBM->SBUF as bf16 via casting SWDGE (gpsimd) DMAs -- no engine
  pass for the conversion, half the SBUF residency of f32.
  phase A (per 512-col tile j): pl = Wk^T x (PE, bf16), E = exp(pl) (ACT,
      with per-tile row-sum accumulators)
  stats: r = sum_N E, rr = 1/r (DVE), Wv' = Wv^T * rr (bf16)
  chains (all j of a batch back-to-back, so ACT Exp<->Recip table swaps are
      batched: 3 total instead of one per j):
      cs = rr^T E (PE), rcs = 1/cs (ACT raw table recip, bf16 out),
      bc = partition_broadcast(rcs) (GPSIMD), E' = E*bc (DVE, 2x bf16 mode)
  phase B (per j-pair, interleaved with phase A of the next batch to keep
      the PE and DMA continuously busy):
      po[128,1024] = Wv'^T E' (PE, bf16, two PSUM banks),
      y = po + x_bf16 (DVE), y -> HBM (HWDGE on SP)

End-to-end relative L2 error vs the fp32 reference: ~1e-3 (bf16 x add).
"""
from contextlib import ExitStack

import numpy as np

import concourse.bacc as bacc
import concourse.mybir as mybir
import concourse.tile as tile
from concourse.bass_utils import run_bass_kernel_spmd

F32 = mybir.dt.float32
BF16 = mybir.dt.bfloat16
AF = mybir.ActivationFunctionType
ALU = mybir.AluOpType
AX = mybir.AxisListType

B, C, M, N = 16, 512, 256, 4096
NCORES = 8
BPC = B // NCORES
NT = 512
NJ = N // NT          # 8 column tiles
KC = C // 128         # 4
KM = M // 128         # 2
XH = 1024             # x load chunk width (one chunk covers 2 j tiles)
NH = N // XH          # 4


def _build(nc):
    x_d = nc.dram_tensor("x", [BPC, KC, 128, N], F32, kind="ExternalInput").ap()
    wkT_d = nc.dram_tensor("wkT", [C, M], F32, kind="ExternalInput").ap()
    msk_d = nc.dram_tensor("msk", [128, 64], F32, kind="ExternalInput").ap()
    wvT_d = nc.dram_tensor("wvT", [M, C], F32, kind="ExternalInput").ap()
    y_d = nc.dram_tensor("y", [BPC, KC, 128, N], F32, kind="ExternalOutput").ap()

    with tile.TileContext(nc) as tc, ExitStack() as ctx:
        F32R = mybir.dt.float32r
        wpool = ctx.enter_context(tc.tile_pool(name="w", bufs=1))
        xbpool = ctx.enter_context(tc.tile_pool(name="xb", bufs=1))
        xpool = ctx.enter_context(tc.tile_pool(name="xp", bufs=2 * NH - 1))
        epool = ctx.enter_context(tc.tile_pool(name="ep", bufs=2 * KM))
        eppool = ctx.enter_context(tc.tile_pool(name="epp", bufs=9))
        spool = ctx.enter_context(tc.tile_pool(name="sp", bufs=4))
        wvp_pool = ctx.enter_context(tc.tile_pool(name="wvp", bufs=2 * KM))
        ypool = ctx.enter_context(tc.tile_pool(name="yp", bufs=4))
        lcpool = ctx.enter_context(tc.tile_pool(name="lcp", bufs=2))
        bcpool = ctx.enter_context(tc.tile_pool(name="bcp", bufs=6))
        rcfpool = ctx.enter_context(tc.tile_pool(name="rcf", bufs=2))
        ps_l = ctx.enter_context(tc.tile_pool(name="ps_l", bufs=2, space="PSUM"))
        ps_cs = ctx.enter_context(tc.tile_pool(name="ps_cs", bufs=2, space="PSUM"))
        ps_o = ctx.enter_context(tc.tile_pool(name="ps_o", bufs=2, space="PSUM"))

        X, E, RSP, RRE, WVP, EPT = {}, {}, {}, {}, {}, {}

        def load_x(b):
            # one casting DMA per XH-wide chunk: [128, KC, XH] bf16.
            # The very first chunk goes over HWDGE as f32 instead: SWDGE has
            # ~8us of descriptor-generation startup latency, and the head of
            # the pipeline (first MM1) is the kernel's critical path.
            X[b] = []
            for h in range(NH):
                src = x_d[b, :, :, h * XH:(h + 1) * XH].rearrange("k p n -> p k n")
                if b == 0 and h == 0:
                    # two half-width HWDGE transfers so MM1(j0) can start as
                    # soon as the first 512 columns land
                    t = xbpool.tile([128, KC, XH], F32R, tag="xtf", name="x0_0")
                    for u in range(2):
                        nc.sync.dma_start(
                            t[:, :, u * NT:(u + 1) * NT],
                            src[:, :, u * NT:(u + 1) * NT].bitcast(F32R))
                else:
                    t = xpool.tile([128, KC, XH], BF16, tag="xt", name=f"x{b}_{h}")
                    nc.gpsimd.dma_start(t[:], src)
                X[b].append(t)

        load_x(0)
        load_x(1)

        # weights ride the second HWDGE queue (ACT engine) so they don't
        # serialize behind the x boot chunk on the SP queue
        wk_sb, wkf_sb = [], []
        for kc in range(KC):
            f = wpool.tile([128, M], F32R, tag=f"wkf{kc}", name=f"wkf{kc}")
            nc.scalar.dma_start(f[:], wkT_d[kc * 128:(kc + 1) * 128, :].bitcast(F32R))
            t = wpool.tile([128, M], BF16, tag=f"wk{kc}", name=f"wk{kc}")
            nc.vector.tensor_copy(t[:], f[:].bitcast(F32))
            wk_sb.append(t)
            wkf_sb.append(f)
        wv_sb = []
        for km in range(KM):
            t = wpool.tile([128, C], F32, tag=f"wv{km}", name=f"wv{km}")
            nc.scalar.dma_start(t[:], wvT_d[km * 128:(km + 1) * 128, :])
            wv_sb.append(t)
        mskf = wpool.tile([128, 8 * NJ], F32, tag="mskf", name="mskf")
        nc.scalar.dma_start(mskf[:], msk_d[:, :])
        msk_sb = wpool.tile([128, 8 * NJ], BF16, tag="msk", name="msk")
        nc.vector.tensor_copy(msk_sb[:], mskf[:])

        def xs(b, kc, j):
            h, jj = j // (XH // NT), j % (XH // NT)
            return X[b][h][:, kc, jj * NT:(jj + 1) * NT]

        def wk_lhsT(b, j, kc, km):
            # match the PE datapath of the rhs: f32r for the f32r boot chunk
            if X[b][j // (XH // NT)].dtype == F32R:
                return wkf_sb[kc][:, km * 128:(km + 1) * 128]
            return wk_sb[kc][:, km * 128:(km + 1) * 128]

        def init_A(b):
            E[b] = [epool.tile([128, N], BF16, tag="e", name=f"e{b}_{km}")
                    for km in range(KM)]
            RSP[b] = [spool.tile([128, NJ], F32, tag="rsp", name=f"rsp{b}_{km}")
                      for km in range(KM)]

        def emit_A(b, j):
            for km in range(KM):
                pl = ps_l.tile([128, NT], F32, tag="pl", name=f"pl{b}_{j}_{km}")
                for kc in range(KC):
                    nc.tensor.matmul(pl[:], wk_lhsT(b, j, kc, km),
                                     xs(b, kc, j),
                                     start=(kc == 0), stop=(kc == KC - 1))
                nc.scalar.activation(E[b][km][:, j * NT:(j + 1) * NT], pl[:],
                                     AF.Exp, accum_out=RSP[b][km][:, j:j + 1])

        def emit_stats(b):
            RRE[b], WVP[b] = [], []
            for km in range(KM):
                rs = spool.tile([128, 1], F32, tag="rs", name=f"rs{b}_{km}")
                nc.vector.tensor_reduce(rs[:], RSP[b][km][:], axis=AX.X, op=ALU.add)
                rr = spool.tile([128, 1], F32, tag="rr", name=f"rr{b}_{km}")
                nc.vector.reciprocal(rr[:], rs[:])
                # rr masked into 8 one-hot column blocks: block j of rr8 is
                # rr in column j, zeros elsewhere -- the cs8 matmul lhsT
                rr8 = spool.tile([128, 8 * NJ], BF16, tag="rr8", name=f"rr8{b}_{km}")
                nc.vector.tensor_scalar_mul(rr8[:], msk_sb[:], rr[:])
                RRE[b].append(rr8)
                t = wvp_pool.tile([128, C], BF16, tag="wvp", name=f"wvp{b}_{km}")
                nc.vector.tensor_scalar_mul(t[:], wv_sb[km][:], rr[:])
                WVP[b].append(t)

        RCF = {}

        def emit_cs(b):
            # all 8 column-tile sums land in one [8, NT] PSUM tile: the
            # masked lhsT routes tile j's colsum to partition row j, so the
            # 1/cs normalization is ONE Ln + ONE Exp instead of 8+8.
            cs8 = ps_cs.tile([8, NT], F32, tag="cs", name=f"cs{b}")
            n = 0
            for j in range(NJ):
                for km in range(KM):
                    nc.tensor.matmul(cs8[:], RRE[b][km][:, j * 8:(j + 1) * 8],
                                     E[b][km][:, j * NT:(j + 1) * NT],
                                     start=(n == 0), stop=(n == 2 * NJ - 1))
                    n += 1
            # 1/cs as exp(-ln(cs)): Ln and Exp share one ACT table
            # (natural_log_exp_and_others), so no table swaps mid-kernel.
            lcs = lcpool.tile([8, NT], F32, tag="lcs", name=f"lcs{b}")
            nc.scalar.activation(lcs[:], cs8[:], AF.Ln)
            rcs = bcpool.tile([8, NT], BF16, tag="rcs", name=f"rcs{b}")
            nc.scalar.activation(rcs[:], lcs[:], AF.Exp, scale=-1.0)
            # flatten the 8 rows to [1, N] on partition 0 with tiny
            # SBUF->SBUF row DMAs so partition_broadcast can source any
            # column range from partition 0
            rcf = rcfpool.tile([1, N], BF16, tag="rcf", name=f"rcf{b}")
            for j in range(NJ):
                nc.sync.dma_start(rcf[0:1, j * NT:(j + 1) * NT],
                                  rcs[j:j + 1, :])
            RCF[b] = rcf

        def emit_chain(b, p):
            # one wide broadcast + two wide E'-mults per j-pair
            cols = slice(p * 2 * NT, (p + 1) * 2 * NT)
            bc = bcpool.tile([128, 2 * NT], BF16, tag="bc", name=f"bc{b}_{p}")
            nc.gpsimd.partition_broadcast(bc[:], RCF[b][0:1, cols])
            ep_t = []
            for km in range(KM):
                t = eppool.tile([128, 2 * NT], BF16, tag="epp",
                                name=f"epp{b}_{p}_{km}")
                nc.vector.tensor_tensor(t[:], E[b][km][:, cols], bc[:],
                                        op=ALU.mult)
                ep_t.append(t)
            EPT[(b, p)] = ep_t

        def emit_mm2_pair(b, p):
            # MM2 + residual add + store for the j-pair p (columns
            # [2p*NT, (2p+2)*NT)); h == p since XH == 2*NT
            ep_t = EPT.pop((b, p))
            for co in range(KC):
                po = ps_o.tile([128, 2 * NT], F32, tag="po", name=f"po{b}_{p}_{co}")
                for jj in range(2):
                    for km in range(KM):
                        nc.tensor.matmul(po[:, jj * NT:(jj + 1) * NT],
                                         WVP[b][km][:, co * 128:(co + 1) * 128],
                                         ep_t[km][:, jj * NT:(jj + 1) * NT],
                                         start=(km == 0), stop=(km == KM - 1))
                xa = X[b][p][:, co, :]
                if xa.dtype == F32R:
                    xa = xa.bitcast(F32)
                yt = ypool.tile([128, 2 * NT], F32, tag="y", name=f"y{b}_{p}_{co}")
                nc.vector.tensor_tensor(yt[:], po[:], xa, op=ALU.add)
                nc.sync.dma_start(
                    y_d[b, co, :, 2 * p * NT:(2 * p + 2) * NT], yt[:])

        # ---- schedule ----
        init_A(0)
        for j in range(NJ):
            emit_A(0, j)
        emit_stats(0)
        emit_cs(0)
        # MM2 pairs interleave with the chains so DVE alternates E'-mults
        # and y-adds: stores start flowing while x(b1) still loads
        emit_chain(0, 0)
        emit_chain(0, 1)
        emit_mm2_pair(0, 0)
        emit_chain(0, 2)
        emit_mm2_pair(0, 1)
        emit_chain(0, 3)
        init_A(1)
        for j in range(NJ):
            emit_A(1, j)
            if j == 3:
                emit_mm2_pair(0, 2)
        emit_stats(1)
        emit_cs(1)
        # last b0 pair is PE filler for the b1 chain latency
        emit_chain(1, 0)
        emit_mm2_pair(0, 3)
        emit_chain(1, 1)
        emit_mm2_pair(1, 0)
        emit_chain(1, 2)
        emit_mm2_pair(1, 1)
        emit_chain(1, 3)
        emit_mm2_pair(1, 2)
        emit_mm2_pair(1, 3)
    return nc


_CACHE = {}


def _steer_act_tables():
    """Make the act-table placement pass resolve both Exp and Ln to the one
    table that holds them both (natural_log_exp_and_others), instead of
    thrashing between exp_and_others and natural_log on every chain.

    Only the *advertised* function sets of the two greedy-first tables are
    filtered; list order (and hence act_func_set_id numbering) is untouched,
    so the runtime still loads real, correct tables.
    """
    from concourse import hw_specs

    orig = hw_specs.get_activation_tables

    def patched(arch):
        tabs = dict(orig(arch))
        exp_f = mybir.ActivationFunctionType.Exp
        ln_f = mybir.ActivationFunctionType.Ln
        both = {n for n, s in tabs.items() if exp_f in s and ln_f in s}
        if both:
            tabs = {n: (s - {exp_f, ln_f} if n not in both else s)
                    for n, s in tabs.items()}
        return tabs

    bacc.get_activation_tables = patched
    return orig


def _get_program():
    if "nc" not in _CACHE:
        nc = bacc.Bacc("TRN2", target_bir_lowering=False, debug=False,
                       enable_asserts=True)
        _build(nc)
        orig = _steer_act_tables()
        try:
            nc.compile()
        finally:
            bacc.get_activation_tables = orig
        _CACHE["nc"] = nc
    return _CACHE["nc"]


def _in_maps(x, Wk, Wv):
    x = np.ascontiguousarray(np.asarray(x), dtype=np.float32)
    wkT = np.ascontiguousarray(np.asarray(Wk, dtype=np.float32).T)
    wvT = np.ascontiguousarray(np.asarray(Wv, dtype=np.float32).T)
    msk = np.ascontiguousarray(
        np.tile(np.eye(8, dtype=np.float32).reshape(1, 64), (128, 1)))
    xs = x.reshape(NCORES, BPC, KC, 128, N)
    return [{"x": xs[i], "wkT": wkT, "wvT": wvT, "msk": msk}
            for i in range(NCORES)]


def kernel(x, Wk, Wv):
    nc = _get_program()
    res = run_bass_kernel_spmd(nc, _in_maps(x, Wk, Wv), list(range(NCORES)))
    y = np.concatenate([res.results[i]["y"].reshape(BPC, C, N)
                        for i in range(NCORES)], axis=0)
    return np.ascontiguousarray(y, dtype=np.float32)


# revision 27
# speedup vs baseline: 1.2028x; 1.2028x over previous
"""External Attention (nn_External_Attention) on 8 TRN2 NeuronCores.

kernel(x, Wk, Wv) -> x + Wv @ l1norm_M(softmax_N(Wk @ x))
  x  [16, 512, 4096] f32,  Wk [256, 512] f32,  Wv [512, 256] f32

Sharding: data-parallel over batch B=16 -> 2 batches per core across 8 cores.
Each core runs an identical Bass/Tile program on its batch shard; results are
concatenated on host.

Per-core pipeline (C=512, M=256, N=4096), all-bf16 on the PE:
  x is loaded HBM->SBUF as bf16 via casting SWDGE (gpsimd) DMAs -- no engine
  pass for the conversion, half the SBUF residency of f32.
  phase A (per 512-col tile j): pl = Wk^T x (PE, bf16), E = exp(pl) (ACT,
      with per-tile row-sum accumulators)
  stats: r = sum_N E, rr = 1/r (DVE), Wv' = Wv^T * rr (bf16)
  chains (all j of a batch back-to-back, so ACT Exp<->Recip table swaps are
      batched: 3 total instead of one per j):
      cs = rr^T E (PE), rcs = 1/cs (ACT raw table recip, bf16 out),
      bc = partition_broadcast(rcs) (GPSIMD), E' = E*bc (DVE, 2x bf16 mode)
  phase B (per j-pair, interleaved with phase A of the next batch to keep
      the PE and DMA continuously busy):
      po[128,1024] = Wv'^T E' (PE, bf16, two PSUM banks),
      y = po + x_bf16 (DVE), y -> HBM (HWDGE on SP)

End-to-end relative L2 error vs the fp32 reference: ~1e-3 (bf16 x add).
"""
from contextlib import ExitStack

import numpy as np

import concourse.bacc as bacc
import concourse.mybir as mybir
import concourse.tile as tile
from concourse.bass_utils import run_bass_kernel_spmd

F32 = mybir.dt.float32
BF16 = mybir.dt.bfloat16
AF = mybir.ActivationFunctionType
ALU = mybir.AluOpType
AX = mybir.AxisListType

B, C, M, N = 16, 512, 256, 4096
NCORES = 8
BPC = B // NCORES
NT = 512
NJ = N // NT          # 8 column tiles
KC = C // 128         # 4
KM = M // 128         # 2
XH = 1024             # x load chunk width (one chunk covers 2 j tiles)
NH = N // XH          # 4


def _build(nc):
    x_d = nc.dram_tensor("x", [BPC, KC, 128, N], F32, kind="ExternalInput").ap()
    wkT_d = nc.dram_tensor("wkT", [C, M], F32, kind="ExternalInput").ap()
    wvT_d = nc.dram_tensor("wvT", [M, C], F32, kind="ExternalInput").ap()
    y_d = nc.dram_tensor("y", [BPC, KC, 128, N], F32, kind="ExternalOutput").ap()

    with tile.TileContext(nc) as tc, ExitStack() as ctx:
        F32R = mybir.dt.float32r
        wpool = ctx.enter_context(tc.tile_pool(name="w", bufs=1))
        xbpool = ctx.enter_context(tc.tile_pool(name="xb", bufs=1))
        xpool = ctx.enter_context(tc.tile_pool(name="xp", bufs=2 * NH - 1))
        epool = ctx.enter_context(tc.tile_pool(name="ep", bufs=2 * KM))
        eppool = ctx.enter_context(tc.tile_pool(name="epp", bufs=9))
        spool = ctx.enter_context(tc.tile_pool(name="sp", bufs=4))
        wvp_pool = ctx.enter_context(tc.tile_pool(name="wvp", bufs=2 * KM))
        ypool = ctx.enter_context(tc.tile_pool(name="yp", bufs=4))
        lcpool = ctx.enter_context(tc.tile_pool(name="lcp", bufs=2))
        bcpool = ctx.enter_context(tc.tile_pool(name="bcp", bufs=6))
        rcfpool = ctx.enter_context(tc.tile_pool(name="rcf", bufs=2))
        ps_l = ctx.enter_context(tc.tile_pool(name="ps_l", bufs=2, space="PSUM"))
        ps_cs = ctx.enter_context(tc.tile_pool(name="ps_cs", bufs=1, space="PSUM"))
        ps_o = ctx.enter_context(tc.tile_pool(name="ps_o", bufs=2, space="PSUM"))

        X, E, RSP, RRE, WVP, EPT = {}, {}, {}, {}, {}, {}

        def load_x(b):
            # one casting DMA per XH-wide chunk: [128, KC, XH] bf16.
            # The very first chunk goes over HWDGE as f32 instead: SWDGE has
            # ~8us of descriptor-generation startup latency, and the head of
            # the pipeline (first MM1) is the kernel's critical path.
            X[b] = []
            for h in range(NH):
                src = x_d[b, :, :, h * XH:(h + 1) * XH].rearrange("k p n -> p k n")
                if b == 0 and h == 0:
                    # two half-width HWDGE transfers so MM1(j0) can start as
                    # soon as the first 512 columns land
                    t = xbpool.tile([128, KC, XH], F32R, tag="xtf", name="x0_0")
                    for u in range(2):
                        nc.sync.dma_start(
                            t[:, :, u * NT:(u + 1) * NT],
                            src[:, :, u * NT:(u + 1) * NT].bitcast(F32R))
                else:
                    t = xpool.tile([128, KC, XH], BF16, tag="xt", name=f"x{b}_{h}")
                    nc.gpsimd.dma_start(t[:], src)
                X[b].append(t)

        load_x(0)
        load_x(1)

        # weights ride the second HWDGE queue (ACT engine) so they don't
        # serialize behind the x boot chunk on the SP queue
        wk_sb, wkf_sb = [], []
        for kc in range(KC):
            f = wpool.tile([128, M], F32R, tag=f"wkf{kc}", name=f"wkf{kc}")
            nc.scalar.dma_start(f[:], wkT_d[kc * 128:(kc + 1) * 128, :].bitcast(F32R))
            t = wpool.tile([128, M], BF16, tag=f"wk{kc}", name=f"wk{kc}")
            nc.vector.tensor_copy(t[:], f[:].bitcast(F32))
            wk_sb.append(t)
            wkf_sb.append(f)
        wv_sb = []
        for km in range(KM):
            t = wpool.tile([128, C], F32, tag=f"wv{km}", name=f"wv{km}")
            nc.scalar.dma_start(t[:], wvT_d[km * 128:(km + 1) * 128, :])
            wv_sb.append(t)

        def xs(b, kc, j):
            h, jj = j // (XH // NT), j % (XH // NT)
            return X[b][h][:, kc, jj * NT:(jj + 1) * NT]

        def wk_lhsT(b, j, kc, km):
            # match the PE datapath of the rhs: f32r for the f32r boot chunk
            if X[b][j // (XH // NT)].dtype == F32R:
                return wkf_sb[kc][:, km * 128:(km + 1) * 128]
            return wk_sb[kc][:, km * 128:(km + 1) * 128]

        def init_A(b):
            E[b] = [epool.tile([128, N], BF16, tag="e", name=f"e{b}_{km}")
                    for km in range(KM)]
            RSP[b] = [spool.tile([128, NJ], F32, tag="rsp", name=f"rsp{b}_{km}")
                      for km in range(KM)]

        def emit_A(b, j):
            for km in range(KM):
                pl = ps_l.tile([128, NT], F32, tag="pl", name=f"pl{b}_{j}_{km}")
                for kc in range(KC):
                    nc.tensor.matmul(pl[:], wk_lhsT(b, j, kc, km),
                                     xs(b, kc, j),
                                     start=(kc == 0), stop=(kc == KC - 1))
                nc.scalar.activation(E[b][km][:, j * NT:(j + 1) * NT], pl[:],
                                     AF.Exp, accum_out=RSP[b][km][:, j:j + 1])

        def emit_stats(b):
            RRE[b], WVP[b] = [], []
            for km in range(KM):
                rs = spool.tile([128, 1], F32, tag="rs", name=f"rs{b}_{km}")
                nc.vector.tensor_reduce(rs[:], RSP[b][km][:], axis=AX.X, op=ALU.add)
                rr = spool.tile([128, 1], F32, tag="rr", name=f"rr{b}_{km}")
                nc.vector.reciprocal(rr[:], rs[:])
                rrb = spool.tile([128, 1], BF16, tag="rrb", name=f"rrb{b}_{km}")
                nc.vector.tensor_copy(rrb[:], rr[:])
                RRE[b].append(rrb)
                t = wvp_pool.tile([128, C], BF16, tag="wvp", name=f"wvp{b}_{km}")
                nc.vector.tensor_scalar_mul(t[:], wv_sb[km][:], rr[:])
                WVP[b].append(t)

        def emit_chain(b, p):
            # colsum for the j-pair lands in one [1, 2*NT] PSUM tile that
            # spans two banks (one accumulation group per NT half), so the
            # 1/cs normalization is one Ln + one Exp + one wide broadcast
            # per pair, all sourced from partition 0.
            cols = slice(p * 2 * NT, (p + 1) * 2 * NT)
            cs2 = ps_cs.tile([1, 2 * NT], F32, tag="cs", name=f"cs{b}_{p}")
            for jj in range(2):
                j = 2 * p + jj
                for km in range(KM):
                    nc.tensor.matmul(cs2[0:1, jj * NT:(jj + 1) * NT],
                                     RRE[b][km][:],
                                     E[b][km][:, j * NT:(j + 1) * NT],
                                     start=(km == 0), stop=(km == KM - 1))
            # 1/cs as exp(-ln(cs)): Ln and Exp share one ACT table
            # (natural_log_exp_and_others), so no table swaps mid-kernel.
            lcs = lcpool.tile([1, 2 * NT], F32, tag="lcs", name=f"lcs{b}_{p}")
            nc.scalar.activation(lcs[:], cs2[:], AF.Ln)
            rcs = bcpool.tile([1, 2 * NT], BF16, tag="rcs", name=f"rcs{b}_{p}")
            nc.scalar.activation(rcs[:], lcs[:], AF.Exp, scale=-1.0)
            bc = bcpool.tile([128, 2 * NT], BF16, tag="bc", name=f"bc{b}_{p}")
            nc.gpsimd.partition_broadcast(bc[:], rcs[:])
            ep_t = []
            for km in range(KM):
                t = eppool.tile([128, 2 * NT], BF16, tag="epp",
                                name=f"epp{b}_{p}_{km}")
                nc.vector.tensor_tensor(t[:], E[b][km][:, cols], bc[:],
                                        op=ALU.mult)
                ep_t.append(t)
            EPT[(b, p)] = ep_t

        def emit_mm2_pair(b, p):
            # MM2 + residual add + store for the j-pair p (columns
            # [2p*NT, (2p+2)*NT)); h == p since XH == 2*NT
            ep_t = EPT.pop((b, p))
            for co in range(KC):
                po = ps_o.tile([128, 2 * NT], F32, tag="po", name=f"po{b}_{p}_{co}")
                for jj in range(2):
                    for km in range(KM):
                        nc.tensor.matmul(po[:, jj * NT:(jj + 1) * NT],
                                         WVP[b][km][:, co * 128:(co + 1) * 128],
                                         ep_t[km][:, jj * NT:(jj + 1) * NT],
                                         start=(km == 0), stop=(km == KM - 1))
                xa = X[b][p][:, co, :]
                if xa.dtype == F32R:
                    xa = xa.bitcast(F32)
                yt = ypool.tile([128, 2 * NT], F32, tag="y", name=f"y{b}_{p}_{co}")
                nc.vector.tensor_tensor(yt[:], po[:], xa, op=ALU.add)
                nc.sync.dma_start(
                    y_d[b, co, :, 2 * p * NT:(2 * p + 2) * NT], yt[:])

        # ---- schedule ----
        init_A(0)
        for j in range(NJ):
            emit_A(0, j)
        emit_stats(0)
        # MM2 pairs interleave with the chains so DVE alternates E'-mults
        # and y-adds: stores start flowing while x(b1) still loads
        emit_chain(0, 0)
        emit_chain(0, 1)
        emit_mm2_pair(0, 0)
        emit_chain(0, 2)
        emit_mm2_pair(0, 1)
        emit_chain(0, 3)
        init_A(1)
        for j in range(NJ):
            emit_A(1, j)
            if j == 3:
                emit_mm2_pair(0, 2)
        emit_stats(1)
        # last b0 pair is PE filler for the b1 chain latency
        emit_chain(1, 0)
        emit_mm2_pair(0, 3)
        emit_chain(1, 1)
        emit_mm2_pair(1, 0)
        emit_chain(1, 2)
        emit_mm2_pair(1, 1)
        emit_chain(1, 3)
        emit_mm2_pair(1, 2)
        emit_mm2_pair(1, 3)
    return nc


_CACHE = {}


def _steer_act_tables():
    """Make the act-table placement pass resolve both Exp and Ln to the one
    table that holds them both (natural_log_exp_and_others), instead of
    thrashing between exp_and_others and natural_log on every chain.

    Only the *advertised* function sets of the two greedy-first tables are
    filtered; list order (and hence act_func_set_id numbering) is untouched,
    so the runtime still loads real, correct tables.
    """
    from concourse import hw_specs

    orig = hw_specs.get_activation_tables

    def patched(arch):
        tabs = dict(orig(arch))
        exp_f = mybir.ActivationFunctionType.Exp
        ln_f = mybir.ActivationFunctionType.Ln
        both = {n for n, s in tabs.items() if exp_f in s and ln_f in s}
        if both:
            tabs = {n: (s - {exp_f, ln_f} if n not in both else s)
                    for n, s in tabs.items()}
        return tabs

    bacc.get_activation_tables = patched
    return orig


def _get_program():
    if "nc" not in _CACHE:
        nc = bacc.Bacc("TRN2", target_bir_lowering=False, debug=False,
                       enable_asserts=True)
        _build(nc)
        orig = _steer_act_tables()
        try:
            nc.compile()
        finally:
            bacc.get_activation_tables = orig
        _CACHE["nc"] = nc
    return _CACHE["nc"]


def _in_maps(x, Wk, Wv):
    x = np.ascontiguousarray(np.asarray(x), dtype=np.float32)
    wkT = np.ascontiguousarray(np.asarray(Wk, dtype=np.float32).T)
    wvT = np.ascontiguousarray(np.asarray(Wv, dtype=np.float32).T)
    xs = x.reshape(NCORES, BPC, KC, 128, N)
    return [{"x": xs[i], "wkT": wkT, "wvT": wvT} for i in range(NCORES)]


def kernel(x, Wk, Wv):
    nc = _get_program()
    res = run_bass_kernel_spmd(nc, _in_maps(x, Wk, Wv), list(range(NCORES)))
    y = np.concatenate([res.results[i]["y"].reshape(BPC, C, N)
                        for i in range(NCORES)], axis=0)
    return np.ascontiguousarray(y, dtype=np.float32)
